# revision 13
# baseline (speedup 1.0000x reference)
"""Trainium2 Bass kernel for DoubleHeadRNN (two independent GRUs over the same input).

Problem: x [64, 1024, 512]; two Keras-style GRUCells (reset_after=True) with
H=1024, T=1024 steps; returns (h_last_head0, h_last_head1).

Strategy (v4): one head per core (cores 0/1 produce the two heads; the SPMD
program is identical on all 8 cores). Per step the fused projection
g = [x_t; h] @ [W; U] runs as PE matmuls with h kept transposed (regenerated
each step by PE transposes). The candidate gate needs xh and hh separately
(h_cand = tanh(xh + r*hh)), so PSUM keeps [zneg | r | hh] + xh regions.
z columns are negated on host so one sigmoid yields zneg = 1-z directly:
    h_new = h + zneg * (cand - h)

v4 critical-path structure (the v2 baseline let the PE idle ~2.6us/step
waiting on the ACT/DVE gate chain before transposing):
 - the transposes of h(i-1) are emitted AFTER step i's half-0 x-only
   matmuls, so the previous step's gate chain completes in their shadow;
 - within each x phase the z/r matmuls come first and the xh matmuls last,
   so the single-buffered xh bank is not overwritten before the previous
   half's t2-add has read it;
 - r's sigmoid is issued before z's (r heads the critical chain).
"""

import os
import numpy as np
from contextlib import ExitStack

B, T, D, H = 64, 1024, 512, 1024
KC = (D + H) // 128  # 12 K-chunks of the fused contraction
NCORES = 8

_cache = {}


def _build(n_steps, bf16=False):
    import concourse.bass as bass
    import concourse.tile as tile
    from concourse import bacc, mybir

    f32 = mybir.dt.float32
    r32 = mybir.dt.float32r
    # float32r: same 4-byte storage, PE streams 1 cycle/row vs fp32's 4.
    mdt = mybir.dt.bfloat16 if bf16 else r32
    AF = mybir.ActivationFunctionType

    nc = bacc.Bacc(
        "TRN2", target_bir_lowering=False, debug=False, num_devices=NCORES
    )
    xt_d = nc.dram_tensor("xt", [n_steps * 128, 256], mdt, kind="ExternalInput").ap()
    wu_d = nc.dram_tensor("wu", [KC * 128, 3072], mdt, kind="ExternalInput").ap()
    id_d = nc.dram_tensor("ident", [128, 64], f32, kind="ExternalInput").ap()
    out_d = nc.dram_tensor("out", [64, 1024], f32, kind="ExternalOutput").ap()

    with tile.TileContext(nc) as tc, ExitStack() as ctx:
        const = ctx.enter_context(tc.tile_pool(name="const", bufs=1))
        state = ctx.enter_context(tc.tile_pool(name="state", bufs=1))
        xpool = ctx.enter_context(tc.tile_pool(name="xin", bufs=4))
        gates = ctx.enter_context(tc.tile_pool(name="gates", bufs=3))
        ppool = ctx.enter_context(tc.tile_pool(name="psum", bufs=2, space="PSUM"))
        xpsum = ctx.enter_context(tc.tile_pool(name="psumX", bufs=1, space="PSUM"))
        tpool = ctx.enter_context(tc.tile_pool(name="psumT", bufs=1, space="PSUM"))

        # --- persistent SBUF ---
        wu_s = const.tile([128, KC * 3072], mdt, tag="wu")
        for c in range(KC):
            nc.sync.dma_start(
                wu_s[:, c * 3072 : (c + 1) * 3072],
                wu_d[c * 128 : (c + 1) * 128, :],
            )
        ident = const.tile([128, 64], f32, tag="ident")
        nc.sync.dma_start(ident[:], id_d[:])

        # h state parity pairs (distance-2 reuse keeps staggered_reset happy)
        h_cur = [state.tile([64, 1024], f32, tag=f"hcur{p}", name=f"hcur{p}") for p in range(2)]
        hT = [state.tile([128, 512], mdt, tag=f"hT{p}", name=f"hT{p}") for p in range(2)]

        nc.vector.memset(h_cur[0][:], 0.0)

        def x_mms(xt_t, ps, xh, hf):
            """x-chunk matmuls for one half: z/r first, xh last."""
            for c in range(4):
                lhsT = xt_t[:, c * 64 : (c + 1) * 64]
                cb = c * 3072 + hf * 512
                nc.tensor.matmul(
                    ps[:, 0:512], lhsT, wu_s[:, cb : cb + 512],
                    start=(c == 0), stop=False, skip_group_check=True,
                )
                nc.tensor.matmul(
                    ps[:, 512:1024], lhsT, wu_s[:, cb + 1024 : cb + 1536],
                    start=(c == 0), stop=False, skip_group_check=True,
                )
            for c in range(4):
                lhsT = xt_t[:, c * 64 : (c + 1) * 64]
                cb = c * 3072 + hf * 512
                nc.tensor.matmul(
                    xh[:, 0:512], lhsT, wu_s[:, cb + 2048 : cb + 2560],
                    start=(c == 0), stop=(c == 3), skip_group_check=True,
                )

        def h_mms(hT_t, ps, hf):
            """Recurrent-chunk matmuls for one half, gate-major (r, hh, z):
            the r group closes 16 matmuls early so the sigmoid/t1/t2/tanh/d
            chain overlaps the hh and z passes; only sig_z/e/h_new trail the
            final matmul.  (The per-matmul stationary reload this implies is
            already paid: each f32r matmul self-loads its weights.)"""
            for j in range(8):
                lhsT = hT_t[:, j * 64 : (j + 1) * 64]
                cb = (4 + j) * 3072 + hf * 512
                nc.tensor.matmul(
                    ps[:, 512:1024], lhsT, wu_s[:, cb + 1024 : cb + 1536],
                    start=False, stop=(j == 7), skip_group_check=True,
                )
            for j in range(8):
                lhsT = hT_t[:, j * 64 : (j + 1) * 64]
                cb = (4 + j) * 3072 + hf * 512
                nc.tensor.matmul(
                    ps[:, 1024:1536], lhsT, wu_s[:, cb + 2048 : cb + 2560],
                    start=(j == 0), stop=(j == 7), skip_group_check=True,
                )
            for j in range(8):
                lhsT = hT_t[:, j * 64 : (j + 1) * 64]
                cb = (4 + j) * 3072 + hf * 512
                nc.tensor.matmul(
                    ps[:, 0:512], lhsT, wu_s[:, cb : cb + 512],
                    start=False, stop=(j == 7), skip_group_check=True,
                )

        def gate_chain(ps, xh, h_in_s, h_out_s):
            """ACT/DVE gate math for one half ([64, 512] tensors)."""
            zrs = gates.tile([64, 1024], f32, tag="zrs")
            # r's sigmoid first (it heads the critical chain)
            nc.scalar.activation(zrs[:, 512:1024], ps[:, 512:1024], AF.Sigmoid)
            t1 = gates.tile([64, 512], f32, tag="t1")
            nc.vector.tensor_mul(t1[:], zrs[:, 512:1024], ps[:, 1024:1536])
            nc.scalar.activation(zrs[:, 0:512], ps[:, 0:512], AF.Sigmoid)
            t2 = gates.tile([64, 512], f32, tag="t2")
            nc.vector.tensor_add(t2[:], t1[:], xh[:])
            cand = gates.tile([64, 512], f32, tag="cand")
            nc.scalar.activation(cand[:], t2[:], AF.Tanh)
            d = gates.tile([64, 512], f32, tag="d")
            nc.vector.tensor_sub(d[:], cand[:], h_in_s)
            e = gates.tile([64, 512], f32, tag="e")
            nc.vector.tensor_mul(e[:], zrs[:, 0:512], d[:])
            nc.vector.tensor_add(h_out_s, h_in_s, e[:])

        def step(iv, p):
            """One GRU step reading state parity p, writing parity 1-p."""
            h_in, h_out = h_cur[p], h_cur[1 - p]

            xt_t = xpool.tile([128, 256], mdt, tag="xt")
            nc.sync.dma_start(xt_t[:], xt_d[bass.ds(iv * 128, 128), :])

            # phase 2: transpose h_in -> hT
            pt = tpool.tile([128, 512], f32, tag="pt")
            for k in range(8):
                nc.tensor.transpose(
                    pt[:, k * 64 : (k + 1) * 64],
                    h_in[:, k * 128 : (k + 1) * 128],
                    ident[0:64, :],
                )
            nc.vector.tensor_copy(hT[p][:, 0:256], pt[:, 0:256])
            nc.vector.tensor_copy(hT[p][:, 256:512], pt[:, 256:512])

            ps0 = ppool.tile([64, 1536], f32, tag="ps")
            xh0 = xpsum.tile([64, 512], f32, tag="xh")
            x_mms(xt_t, ps0, xh0, 0)
            # phase 3: recurrent matmuls half0, then gates half0
            h_mms(hT[p], ps0, 0)
            gate_chain(ps0, xh0, h_in[:, 0:512], h_out[:, 0:512])

            # half1 (x + recurrent matmuls cover gates-half0's latency)
            ps1 = ppool.tile([64, 1536], f32, tag="ps")
            xh1 = xpsum.tile([64, 512], f32, tag="xh")
            x_mms(xt_t, ps1, xh1, 1)
            h_mms(hT[p], ps1, 1)
            gate_chain(ps1, xh1, h_in[:, 512:1024], h_out[:, 512:1024])

        with tc.For_i(0, n_steps, 4, hint_engines=(mybir.EngineType.PE,), staggered_reset=True) as i:
            step(i, 0)
            step(i + 1, 1)
            step(i + 2, 0)
            step(i + 3, 1)

        nc.sync.dma_start(out_d[:], h_cur[0][:])

    nc.compile()
    return nc


def _host_prep(x, W, U, bf16=False):
    """Build xt / wu host-side arrays for one head."""
    n_steps = x.shape[1]
    xt = (
        x.transpose(1, 2, 0)                      # [T, D, B]
        .reshape(n_steps, 4, 128, B)              # [T, c, p, b]
        .transpose(0, 2, 1, 3)                    # [T, p, c, b]
        .reshape(n_steps * 128, 256)
        .astype(np.float32)
    )
    Wp = np.asarray(W, np.float32)
    Up = np.asarray(U, np.float32)
    wu = np.concatenate([Wp, Up], axis=0).copy()  # [1536, 3072]
    # negate z columns
    wu[:, 0:H] *= -1.0
    if bf16:
        import ml_dtypes
        xt = xt.astype(ml_dtypes.bfloat16)
        wu = wu.astype(ml_dtypes.bfloat16)
    return np.ascontiguousarray(xt), np.ascontiguousarray(wu)


def _unpermute_h(res):
    """h is stored in natural order."""
    return np.asarray(res, np.float32)


def _make_ident():
    id2 = np.zeros((128, 64), np.float32)
    for p in range(128):
        id2[p, p % 64] = 1.0
    return id2


def _run_spmd(nc, in_maps, n_timed=0):
    """Execute on the 8 axon cores via PJRT shard_map; keeps the jitted
    callable + device inputs resident so timed runs measure execution."""
    import time
    import jax
    from jax.sharding import Mesh, PartitionSpec
    from jax.experimental.shard_map import shard_map
    from concourse import bass2jax, mybir

    bass2jax.install_neuronx_cc_hook()
    n_cores = len(in_maps)

    in_names, out_names, out_avals = [], [], []
    partition_name = nc.partition_id_tensor.name if nc.partition_id_tensor else None
    for alloc in nc.m.functions[0].allocations:
        if not isinstance(alloc, mybir.MemoryLocationSet):
            continue
        name = alloc.memorylocations[0].name
        if alloc.kind == "ExternalInput":
            if name != partition_name:
                in_names.append(name)
        elif alloc.kind == "ExternalOutput":
            shape = tuple(alloc.tensor_shape)
            dtype = mybir.dt.np(alloc.dtype)
            out_avals.append(jax.core.ShapedArray(shape, dtype))
            out_names.append(name)
    n_params = len(in_names)
    n_outs = len(out_names)
    all_in = in_names + out_names
    if partition_name is not None:
        all_in.append(partition_name)

    def _body(*args):
        operands = list(args)
        if partition_name is not None:
            operands.append(bass2jax.partition_id_tensor())
        outs = bass2jax._bass_exec_p.bind(
            *operands,
            out_avals=tuple(out_avals),
            in_names=tuple(all_in),
            out_names=tuple(out_names),
            lowering_input_output_aliases=(),
            sim_require_finite=True,
            sim_require_nnan=True,
            nc=nc,
        )
        return tuple(outs)

    devices = jax.devices()[:n_cores]
    mesh = Mesh(np.asarray(devices), ("core",))
    in_specs = (PartitionSpec("core"),) * (n_params + n_outs)
    out_specs = (PartitionSpec("core"),) * n_outs
    sharded = jax.jit(
        shard_map(_body, mesh=mesh, in_specs=in_specs, out_specs=out_specs,
                  check_rep=False),
        keep_unused=True,
    )
    sharding = jax.sharding.NamedSharding(mesh, PartitionSpec("core"))

    def _stage(per_core_arrays):
        shards = []
        for c, arr in enumerate(per_core_arrays):
            sh = jax.device_put(np.asarray(arr), devices[c])
            sh.block_until_ready()
            shards.append(sh)
        a0 = np.asarray(per_core_arrays[0])
        gshape = (n_cores * a0.shape[0], *a0.shape[1:])
        return jax.make_array_from_single_device_arrays(gshape, sharding, shards)

    dev_in = [_stage([in_maps[c][nm] for c in range(n_cores)]) for nm in in_names]
    dev_zero = [
        _stage([np.zeros(av.shape, av.dtype) for _ in range(n_cores)])
        for av in out_avals
    ]
    for a in dev_in + dev_zero:
        a.block_until_ready()

    out_arrs = sharded(*dev_in, *dev_zero)
    jax.block_until_ready(out_arrs)

    best = None
    for _ in range(n_timed):
        t0 = time.perf_counter_ns()
        out_arrs = sharded(*dev_in, *dev_zero)
        jax.block_until_ready(out_arrs)
        dt = time.perf_counter_ns() - t0
        best = dt if best is None else min(best, dt)

    results = [
        {
            nm: np.asarray(out_arrs[i]).reshape(n_cores, *out_avals[i].shape)[c]
            for i, nm in enumerate(out_names)
        }
        for c in range(n_cores)
    ]
    return results, best


def kernel(x, W0, U0, bi0, br0, W1, U1, bi1, br1):
    x = np.asarray(x, dtype=np.float32)
    assert all(
        not np.any(np.asarray(b)) for b in (bi0, br0, bi1, br1)
    ), "nonzero biases not supported by this kernel build"

    bf16 = bool(int(os.environ.get("GRU_BF16", "0")))
    n_steps = x.shape[1]
    key = (n_steps, bf16)
    if key not in _cache:
        _cache[key] = _build(n_steps, bf16=bf16)
    nc = _cache[key]

    xt, wu0 = _host_prep(x, np.asarray(W0), np.asarray(U0), bf16=bf16)
    _, wu1 = _host_prep(x[:, :1], np.asarray(W1), np.asarray(U1), bf16=bf16)
    ident = _make_ident()

    maps = []
    for core in range(NCORES):
        wu = wu0 if core % 2 == 0 else wu1
        maps.append({"xt": xt, "wu": wu, "ident": ident})

    n_timed = int(os.environ.get("GRU_TIMED_RUNS", "0"))
    results, best_ns = _run_spmd(nc, maps, n_timed=n_timed)
    kernel.last_exec_ns = best_ns
    out0 = _unpermute_h(results[0]["out"])
    out1 = _unpermute_h(results[1]["out"])
    return out0, out1


kernel.last_exec_ns = None


# revision 14
# speedup vs baseline: 1.0950x; 1.0950x over previous
"""Trainium2 Bass kernel for DoubleHeadRNN (two independent GRUs over the same input).

Problem: x [64, 1024, 512]; two Keras-style GRUCells (reset_after=True) with
H=1024, T=1024 steps; returns (h_last_head0, h_last_head1).

Strategy (v4): one head per core (cores 0/1 produce the two heads; the SPMD
program is identical on all 8 cores). Per step the fused projection
g = [x_t; h] @ [W; U] runs as PE matmuls with h kept transposed (regenerated
each step by PE transposes). The candidate gate needs xh and hh separately
(h_cand = tanh(xh + r*hh)), so PSUM keeps [zneg | r | hh] + xh regions.
z columns are negated on host so one sigmoid yields zneg = 1-z directly:
    h_new = h + zneg * (cand - h)

v4 critical-path structure (the v2 baseline let the PE idle ~2.6us/step
waiting on the ACT/DVE gate chain before transposing):
 - the transposes of h(i-1) are emitted AFTER step i's half-0 x-only
   matmuls, so the previous step's gate chain completes in their shadow;
 - within each x phase the z/r matmuls come first and the xh matmuls last,
   so the single-buffered xh bank is not overwritten before the previous
   half's t2-add has read it;
 - r's sigmoid is issued before z's (r heads the critical chain).
"""

import os
import numpy as np
from contextlib import ExitStack

B, T, D, H = 64, 1024, 512, 1024
KC = (D + H) // 128  # 12 K-chunks of the fused contraction
NCORES = 8

_cache = {}


def _build(n_steps, bf16=False):
    import concourse.bass as bass
    import concourse.tile as tile
    from concourse import bacc, mybir

    f32 = mybir.dt.float32
    r32 = mybir.dt.float32r
    # float32r: same 4-byte storage, PE streams 1 cycle/row vs fp32's 4.
    mdt = mybir.dt.bfloat16 if bf16 else r32
    AF = mybir.ActivationFunctionType

    nc = bacc.Bacc(
        "TRN2", target_bir_lowering=False, debug=False, num_devices=NCORES
    )
    xt_d = nc.dram_tensor("xt", [n_steps * 128, 256], mdt, kind="ExternalInput").ap()
    wu_d = nc.dram_tensor("wu", [KC * 128, 3072], mdt, kind="ExternalInput").ap()
    id_d = nc.dram_tensor("ident", [128, 64], f32, kind="ExternalInput").ap()
    out_d = nc.dram_tensor("out", [64, 1024], f32, kind="ExternalOutput").ap()

    with tile.TileContext(nc) as tc, ExitStack() as ctx:
        const = ctx.enter_context(tc.tile_pool(name="const", bufs=1))
        state = ctx.enter_context(tc.tile_pool(name="state", bufs=1))
        xpool = ctx.enter_context(tc.tile_pool(name="xin", bufs=4))
        gates = ctx.enter_context(tc.tile_pool(name="gates", bufs=3))
        ppool = ctx.enter_context(tc.tile_pool(name="psum", bufs=2, space="PSUM"))
        xpsum = ctx.enter_context(tc.tile_pool(name="psumX", bufs=1, space="PSUM"))
        tpool = ctx.enter_context(tc.tile_pool(name="psumT", bufs=1, space="PSUM"))

        # --- persistent SBUF ---
        wu_s = const.tile([128, KC * 3072], mdt, tag="wu")
        for c in range(KC):
            nc.sync.dma_start(
                wu_s[:, c * 3072 : (c + 1) * 3072],
                wu_d[c * 128 : (c + 1) * 128, :],
            )
        ident = const.tile([128, 64], f32, tag="ident")
        nc.sync.dma_start(ident[:], id_d[:])

        # h state parity pairs (distance-2 reuse keeps staggered_reset happy)
        h_cur = [state.tile([64, 1024], f32, tag=f"hcur{p}", name=f"hcur{p}") for p in range(2)]
        hT = [state.tile([128, 512], mdt, tag=f"hT{p}", name=f"hT{p}") for p in range(2)]

        nc.vector.memset(h_cur[0][:], 0.0)

        def x_mms(xt_t, ps, xh, hf):
            """x-chunk matmuls for one half: z/r first, xh last."""
            for c in range(4):
                lhsT = xt_t[:, c * 64 : (c + 1) * 64]
                cb = c * 3072 + hf * 512
                nc.tensor.matmul(
                    ps[:, 0:512], lhsT, wu_s[:, cb : cb + 512],
                    start=(c == 0), stop=False, skip_group_check=True,
                )
                nc.tensor.matmul(
                    ps[:, 512:1024], lhsT, wu_s[:, cb + 1024 : cb + 1536],
                    start=(c == 0), stop=False, skip_group_check=True,
                )
            for c in range(4):
                lhsT = xt_t[:, c * 64 : (c + 1) * 64]
                cb = c * 3072 + hf * 512
                nc.tensor.matmul(
                    xh[:, 0:512], lhsT, wu_s[:, cb + 2048 : cb + 2560],
                    start=(c == 0), stop=(c == 3), skip_group_check=True,
                )

        def h_mms(hT_t, ps, hf):
            """Recurrent-chunk matmuls for one half: r+hh paired per chunk
            (they share one stationary load - walrus dedupes consecutive
            identical lhsT), then a z-only pass.  r/hh close 8 matmuls early,
            so the sigmoid/t1/t2/tanh/d chain overlaps the z pass; only
            sig_z/e/h_new trail the final matmul."""
            for j in range(8):
                lhsT = hT_t[:, j * 64 : (j + 1) * 64]
                cb = (4 + j) * 3072 + hf * 512
                nc.tensor.matmul(
                    ps[:, 512:1024], lhsT, wu_s[:, cb + 1024 : cb + 1536],
                    start=False, stop=(j == 7), skip_group_check=True,
                )
                nc.tensor.matmul(
                    ps[:, 1024:1536], lhsT, wu_s[:, cb + 2048 : cb + 2560],
                    start=(j == 0), stop=(j == 7), skip_group_check=True,
                )
            for j in range(8):
                lhsT = hT_t[:, j * 64 : (j + 1) * 64]
                cb = (4 + j) * 3072 + hf * 512
                nc.tensor.matmul(
                    ps[:, 0:512], lhsT, wu_s[:, cb : cb + 512],
                    start=False, stop=(j == 7), skip_group_check=True,
                )

        def gate_chain(ps, xh, h_in_s, h_out_s):
            """ACT/DVE gate math for one half ([64, 512] tensors)."""
            zrs = gates.tile([64, 1024], f32, tag="zrs")
            # r's sigmoid first (it heads the critical chain)
            nc.scalar.activation(zrs[:, 512:1024], ps[:, 512:1024], AF.Sigmoid)
            t1 = gates.tile([64, 512], f32, tag="t1")
            nc.vector.tensor_mul(t1[:], zrs[:, 512:1024], ps[:, 1024:1536])
            nc.scalar.activation(zrs[:, 0:512], ps[:, 0:512], AF.Sigmoid)
            t2 = gates.tile([64, 512], f32, tag="t2")
            nc.vector.tensor_add(t2[:], t1[:], xh[:])
            cand = gates.tile([64, 512], f32, tag="cand")
            nc.scalar.activation(cand[:], t2[:], AF.Tanh)
            d = gates.tile([64, 512], f32, tag="d")
            nc.vector.tensor_sub(d[:], cand[:], h_in_s)
            e = gates.tile([64, 512], f32, tag="e")
            nc.vector.tensor_mul(e[:], zrs[:, 0:512], d[:])
            nc.vector.tensor_add(h_out_s, h_in_s, e[:])

        def step(iv, p):
            """One GRU step reading state parity p, writing parity 1-p."""
            h_in, h_out = h_cur[p], h_cur[1 - p]

            xt_t = xpool.tile([128, 256], mdt, tag="xt")
            nc.sync.dma_start(xt_t[:], xt_d[bass.ds(iv * 128, 128), :])

            # phase 2: transpose h_in -> hT
            pt = tpool.tile([128, 512], f32, tag="pt")
            for k in range(8):
                nc.tensor.transpose(
                    pt[:, k * 64 : (k + 1) * 64],
                    h_in[:, k * 128 : (k + 1) * 128],
                    ident[0:64, :],
                )
            nc.vector.tensor_copy(hT[p][:, 0:256], pt[:, 0:256])
            nc.vector.tensor_copy(hT[p][:, 256:512], pt[:, 256:512])

            ps0 = ppool.tile([64, 1536], f32, tag="ps")
            xh0 = xpsum.tile([64, 512], f32, tag="xh")
            x_mms(xt_t, ps0, xh0, 0)
            # phase 3: recurrent matmuls half0, then gates half0
            h_mms(hT[p], ps0, 0)
            gate_chain(ps0, xh0, h_in[:, 0:512], h_out[:, 0:512])

            # half1 (x + recurrent matmuls cover gates-half0's latency)
            ps1 = ppool.tile([64, 1536], f32, tag="ps")
            xh1 = xpsum.tile([64, 512], f32, tag="xh")
            x_mms(xt_t, ps1, xh1, 1)
            h_mms(hT[p], ps1, 1)
            gate_chain(ps1, xh1, h_in[:, 512:1024], h_out[:, 512:1024])

        with tc.For_i(0, n_steps, 4, hint_engines=(mybir.EngineType.PE,), staggered_reset=True) as i:
            step(i, 0)
            step(i + 1, 1)
            step(i + 2, 0)
            step(i + 3, 1)

        nc.sync.dma_start(out_d[:], h_cur[0][:])

    nc.compile()
    return nc


def _host_prep(x, W, U, bf16=False):
    """Build xt / wu host-side arrays for one head."""
    n_steps = x.shape[1]
    xt = (
        x.transpose(1, 2, 0)                      # [T, D, B]
        .reshape(n_steps, 4, 128, B)              # [T, c, p, b]
        .transpose(0, 2, 1, 3)                    # [T, p, c, b]
        .reshape(n_steps * 128, 256)
        .astype(np.float32)
    )
    Wp = np.asarray(W, np.float32)
    Up = np.asarray(U, np.float32)
    wu = np.concatenate([Wp, Up], axis=0).copy()  # [1536, 3072]
    # negate z columns
    wu[:, 0:H] *= -1.0
    if bf16:
        import ml_dtypes
        xt = xt.astype(ml_dtypes.bfloat16)
        wu = wu.astype(ml_dtypes.bfloat16)
    return np.ascontiguousarray(xt), np.ascontiguousarray(wu)


def _unpermute_h(res):
    """h is stored in natural order."""
    return np.asarray(res, np.float32)


def _make_ident():
    id2 = np.zeros((128, 64), np.float32)
    for p in range(128):
        id2[p, p % 64] = 1.0
    return id2


def _run_spmd(nc, in_maps, n_timed=0):
    """Execute on the 8 axon cores via PJRT shard_map; keeps the jitted
    callable + device inputs resident so timed runs measure execution."""
    import time
    import jax
    from jax.sharding import Mesh, PartitionSpec
    from jax.experimental.shard_map import shard_map
    from concourse import bass2jax, mybir

    bass2jax.install_neuronx_cc_hook()
    n_cores = len(in_maps)

    in_names, out_names, out_avals = [], [], []
    partition_name = nc.partition_id_tensor.name if nc.partition_id_tensor else None
    for alloc in nc.m.functions[0].allocations:
        if not isinstance(alloc, mybir.MemoryLocationSet):
            continue
        name = alloc.memorylocations[0].name
        if alloc.kind == "ExternalInput":
            if name != partition_name:
                in_names.append(name)
        elif alloc.kind == "ExternalOutput":
            shape = tuple(alloc.tensor_shape)
            dtype = mybir.dt.np(alloc.dtype)
            out_avals.append(jax.core.ShapedArray(shape, dtype))
            out_names.append(name)
    n_params = len(in_names)
    n_outs = len(out_names)
    all_in = in_names + out_names
    if partition_name is not None:
        all_in.append(partition_name)

    def _body(*args):
        operands = list(args)
        if partition_name is not None:
            operands.append(bass2jax.partition_id_tensor())
        outs = bass2jax._bass_exec_p.bind(
            *operands,
            out_avals=tuple(out_avals),
            in_names=tuple(all_in),
            out_names=tuple(out_names),
            lowering_input_output_aliases=(),
            sim_require_finite=True,
            sim_require_nnan=True,
            nc=nc,
        )
        return tuple(outs)

    devices = jax.devices()[:n_cores]
    mesh = Mesh(np.asarray(devices), ("core",))
    in_specs = (PartitionSpec("core"),) * (n_params + n_outs)
    out_specs = (PartitionSpec("core"),) * n_outs
    sharded = jax.jit(
        shard_map(_body, mesh=mesh, in_specs=in_specs, out_specs=out_specs,
                  check_rep=False),
        keep_unused=True,
    )
    sharding = jax.sharding.NamedSharding(mesh, PartitionSpec("core"))

    def _stage(per_core_arrays):
        shards = []
        for c, arr in enumerate(per_core_arrays):
            sh = jax.device_put(np.asarray(arr), devices[c])
            sh.block_until_ready()
            shards.append(sh)
        a0 = np.asarray(per_core_arrays[0])
        gshape = (n_cores * a0.shape[0], *a0.shape[1:])
        return jax.make_array_from_single_device_arrays(gshape, sharding, shards)

    dev_in = [_stage([in_maps[c][nm] for c in range(n_cores)]) for nm in in_names]
    dev_zero = [
        _stage([np.zeros(av.shape, av.dtype) for _ in range(n_cores)])
        for av in out_avals
    ]
    for a in dev_in + dev_zero:
        a.block_until_ready()

    out_arrs = sharded(*dev_in, *dev_zero)
    jax.block_until_ready(out_arrs)

    best = None
    for _ in range(n_timed):
        t0 = time.perf_counter_ns()
        out_arrs = sharded(*dev_in, *dev_zero)
        jax.block_until_ready(out_arrs)
        dt = time.perf_counter_ns() - t0
        best = dt if best is None else min(best, dt)

    results = [
        {
            nm: np.asarray(out_arrs[i]).reshape(n_cores, *out_avals[i].shape)[c]
            for i, nm in enumerate(out_names)
        }
        for c in range(n_cores)
    ]
    return results, best


def kernel(x, W0, U0, bi0, br0, W1, U1, bi1, br1):
    x = np.asarray(x, dtype=np.float32)
    assert all(
        not np.any(np.asarray(b)) for b in (bi0, br0, bi1, br1)
    ), "nonzero biases not supported by this kernel build"

    bf16 = bool(int(os.environ.get("GRU_BF16", "0")))
    n_steps = x.shape[1]
    key = (n_steps, bf16)
    if key not in _cache:
        _cache[key] = _build(n_steps, bf16=bf16)
    nc = _cache[key]

    xt, wu0 = _host_prep(x, np.asarray(W0), np.asarray(U0), bf16=bf16)
    _, wu1 = _host_prep(x[:, :1], np.asarray(W1), np.asarray(U1), bf16=bf16)
    ident = _make_ident()

    maps = []
    for core in range(NCORES):
        wu = wu0 if core % 2 == 0 else wu1
        maps.append({"xt": xt, "wu": wu, "ident": ident})

    n_timed = int(os.environ.get("GRU_TIMED_RUNS", "0"))
    results, best_ns = _run_spmd(nc, maps, n_timed=n_timed)
    kernel.last_exec_ns = best_ns
    out0 = _unpermute_h(results[0]["out"])
    out1 = _unpermute_h(results[1]["out"])
    return out0, out1


kernel.last_exec_ns = None
